# revision 17
# baseline (speedup 1.0000x reference)
"""MBart expert-layer (MoE routing) kernel for 8 Trainium2 NeuronCores.

Strategy: data-parallel over batch. Each batch row routes to exactly one
expert (lang code), so the expert gather happens on host (langs are host
data).  Core b computes a dense SwiGLU MLP for row b:
    out = (gelu(x @ W1) * (x @ W3)) @ W2
All device work happens in transposed orientation (activations stored
[d_model, seq]) so both matmul stages use the natural [K, M] weight layouts
as the stationary operand and no on-device transposes are needed.
Matmul inputs are bf16 (fp32 accumulate in PSUM); gelu/mul in fp32.

DMA wait-budget: this toolchain's walrus rejects DMAs carrying more than
one sync wait.  Weight loads go through the gpsimd SWDGE lanes with buffer
counts sized so a slot is reused exactly 8 DMAs later (same round-robin
lane -> FIFO ordering replaces the WAW wait); x loads and output stores go
through the sync HWDGE lanes where they need at most one RAW wait.
build_nc() asserts the ≤1-wait invariant after scheduling.
"""

import numpy as np
import ml_dtypes
from contextlib import ExitStack

import concourse.bass as bass
import concourse.bacc as bacc
import concourse.mybir as mybir
from concourse.tile import TileContext
from concourse.bass_utils import run_bass_kernel_spmd

E, B, S, D, F = 8, 8, 2048, 1024, 4096
LANG_BASE = 4
P = 128
MT = 512          # matmul moving free dim (seq chunk)
NG = 2            # seq super-chunks; weights streamed NG times
BF16 = mybir.dt.bfloat16
F32 = mybir.dt.float32
bf16 = ml_dtypes.bfloat16


def build_nc(S_=S, D_=D, F_=F, MT_=MT, NG_=NG,
             act=mybir.ActivationFunctionType.Gelu, check_waits=True):
    DT, FT = D_ // P, F_ // P
    sg = S_ // NG_
    nm = sg // MT_
    W2SUB = 4 if FT % 4 == 0 else 1   # w2 block split into sub-DMAs per d_i
    FS = FT // W2SUB                  # f-tiles per w2 sub-block
    nc = bacc.Bacc()
    xt = nc.declare_dram_parameter("xt", [DT, P, S_], BF16, isOutput=False)
    w1 = nc.declare_dram_parameter("w1", [FT, P, DT, P], BF16, isOutput=False)
    w3 = nc.declare_dram_parameter("w3", [FT, P, DT, P], BF16, isOutput=False)
    w2 = nc.declare_dram_parameter("w2", [DT, P, FT, P], BF16, isOutput=False)
    ot = nc.declare_dram_parameter("ot", [DT, P, S_], F32, isOutput=True)

    with TileContext(nc) as tc, ExitStack() as ctx:
        xpool = ctx.enter_context(tc.tile_pool(name="x", bufs=1))
        wpool = ctx.enter_context(tc.tile_pool(name="w", bufs=4))
        w2pool = ctx.enter_context(tc.tile_pool(name="w2", bufs=2))
        hpool = ctx.enter_context(tc.tile_pool(name="h", bufs=1))
        gpool = ctx.enter_context(tc.tile_pool(name="g", bufs=3))
        opool = ctx.enter_context(tc.tile_pool(name="o", bufs=3))
        ppool = ctx.enter_context(tc.tile_pool(name="ps", bufs=2, space="PSUM"))
        p2pool = ctx.enter_context(tc.tile_pool(name="ps2", bufs=2, space="PSUM"))

        x_sb = []
        for d_i in range(DT):
            t = xpool.tile([P, S_], BF16, name=f"x{d_i}", tag=f"x{d_i}")
            nc.sync.dma_start(out=t[:], in_=xt[d_i])
            x_sb.append(t)

        for g in range(NG_):
            s0 = g * sg
            # ---- phase A: hT[f, m] = gelu(W1.T x) * (W3.T x) ----
            h_tiles = []
            for f_i in range(FT):
                w1_t = wpool.tile([P, DT, P], BF16, name="w1t", tag="w1t")
                w3_t = wpool.tile([P, DT, P], BF16, name="w3t", tag="w3t")
                nc.gpsimd.dma_start(out=w1_t[:], in_=w1[f_i])
                nc.gpsimd.dma_start(out=w3_t[:], in_=w3[f_i])
                h_sb = hpool.tile([P, sg], BF16, name=f"h{f_i}", tag=f"h{f_i}")
                for m in range(nm):
                    ms = s0 + m * MT_
                    a_ps = ppool.tile([P, MT_], F32, name="a_ps", tag="a")
                    b_ps = ppool.tile([P, MT_], F32, name="b_ps", tag="b")
                    # b (w3 path) first so the gelu is the latest producer
                    # feeding the h-mul: the wait legalizer can then anchor
                    # the mul's PE wait on the gelu at zero cost.
                    for d_i in range(DT):
                        nc.tensor.matmul(
                            b_ps[:], w3_t[:, d_i, :], x_sb[d_i][:, ms:ms + MT_],
                            start=(d_i == 0), stop=(d_i == DT - 1))
                    for d_i in range(DT):
                        nc.tensor.matmul(
                            a_ps[:], w1_t[:, d_i, :], x_sb[d_i][:, ms:ms + MT_],
                            start=(d_i == 0), stop=(d_i == DT - 1))
                    g_sb = gpool.tile([P, MT_], F32, name="g_sb", tag="g")
                    nc.scalar.activation(g_sb[:], a_ps[:], act)
                    nc.vector.tensor_mul(
                        h_sb[:, m * MT_:(m + 1) * MT_], g_sb[:], b_ps[:])
                h_tiles.append(h_sb)
            # ---- phase B: outT[d, m] = W2.T hT ----
            for d_i in range(DT):
                w2_ts = []
                for k in range(W2SUB):
                    w2_t = w2pool.tile([P, FS, P], BF16, name=f"w2t{k}",
                                       tag=f"w2t{k}")
                    nc.gpsimd.dma_start(
                        out=w2_t[:], in_=w2[d_i][:, k * FS:(k + 1) * FS, :])
                    w2_ts.append(w2_t)
                for m in range(nm):
                    o_ps = p2pool.tile([P, MT_], F32, name="o_ps", tag="o")
                    for f_i in range(FT):
                        nc.tensor.matmul(
                            o_ps[:], w2_ts[f_i // FS][:, f_i % FS, :],
                            h_tiles[f_i][:, m * MT_:(m + 1) * MT_],
                            start=(f_i == 0), stop=(f_i == FT - 1))
                    o_sb = opool.tile([P, MT_], F32, name="o_sb", tag="osb")
                    nc.vector.tensor_copy(o_sb[:], o_ps[:])
                    nc.sync.dma_start(
                        out=ot[d_i][:, s0 + m * MT_:s0 + (m + 1) * MT_],
                        in_=o_sb[:])

    nc.compile()
    if check_waits:
        skip = ("InstDrain", "InstEventSemaphore")
        bad = []
        for f in nc.m.functions:
            for bb in f.blocks:
                for inst in bb.instructions:
                    if type(inst).__name__ in skip or inst.sync_info is None:
                        continue
                    nw = len(inst.sync_info.on_wait or [])
                    if nw > 1:
                        bad.append((inst.name, type(inst).__name__, nw))
        if bad:
            raise RuntimeError(f"insts with >1 wait: {bad[:8]}")
    return nc


def _legalize_dma_waits(nc):
    """The TPB ISA has one wait slot per instruction and this walrus build
    refuses to split multi-wait instructions, so cap every executable
    instruction at one sync wait by moving excess waits onto earlier
    instructions.

    Soundness: a wait moved from X to an instruction Y that provably
    executes before X (same engine queue, or the producer of X's kept
    wait) still gates X because semaphores are monotone.  A moved wait w
    is only placed on Y when pos(producer(w)) < pos(Y) in the scheduled
    (topological) order, so producer(w) cannot transitively depend on Y
    and no wait cycle can form."""
    SKIP = ("InstDrain", "InstEventSemaphore", "InstUnconditionalBranch",
            "InstRegisterMove", "InstCall", "InstISA")
    insts = []
    for f in nc.m.functions:
        for bb in f.blocks:
            insts.extend(bb.instructions)
    pos = {id(i): k for k, i in enumerate(insts)}
    cum = {}
    producer = {}   # (sem_id, cum_value) -> inst
    for i in insts:
        si = i.sync_info
        if si is None:
            continue
        for u in (si.on_update or []):
            if u.update_mode == "sem-inc" and u.update_value is not None:
                cum[u.id] = cum.get(u.id, 0) + u.update_value
                producer[(u.id, cum[u.id])] = i

    def waits_of(i):
        si = i.sync_info
        return list(si.on_wait or []) if si else []

    def set_waits(i, ws):
        si = i.sync_info
        i.sync_info = mybir.SyncInfo(
            on_wait=ws, on_update=list(si.on_update or []) if si else [])

    def add_wait(y, w):
        ws = waits_of(y)
        for x in ws:
            if x.id == w.id:
                if w.wait_value > x.wait_value:
                    ws = [z for z in ws if z.id != w.id] + [w]
                    set_waits(y, ws)
                return
        ws.append(w)
        set_waits(y, ws)

    # same-engine instruction lists for predecessor walk-back
    by_engine = {}
    for i in insts:
        if type(i).__name__ in SKIP or "DMA" in type(i).__name__:
            continue
        by_engine.setdefault(i.engine, []).append(i)

    for _pass in range(6):
        dirty = False
        for i in insts:
            tn = type(i).__name__
            if tn in SKIP:
                continue
            waits = waits_of(i)
            if len(waits) <= 1:
                continue
            dirty = True
            scored = []
            for w in waits:
                p = producer.get((w.id, w.wait_value))
                scored.append((pos[id(p)] if p is not None else -1, w, p))
            scored.sort(key=lambda t: -t[0])
            keep_pos, keep, keep_p = scored[0]
            for wpos, w, wp in scored[1:]:
                placed = False
                # 1) latest 0-wait same-engine predecessor after producer(w)
                cand = [y for y in by_engine.get(i.engine, [])
                        if wpos < pos[id(y)] < pos[id(i)]
                        and not waits_of(y)]
                if cand:
                    y = max(cand, key=lambda y: pos[id(y)])
                    add_wait(y, w)
                    placed = True
                # 2) producer of the kept wait, if later than producer(w)
                elif (keep_p is not None and keep_pos > wpos
                      and "DMA" not in type(keep_p).__name__
                      and type(keep_p).__name__ not in SKIP):
                    add_wait(keep_p, w)
                    placed = True
                if not placed:
                    raise RuntimeError(
                        f"legalize: cannot move wait {w.ant_name}>="
                        f"{w.wait_value} off {i.name} ({tn})")
            set_waits(i, [keep])
        if not dirty:
            break
    else:
        raise RuntimeError("legalize: did not converge")


_NC_CACHE = {}


def _get_nc():
    if "nc" not in _NC_CACHE:
        _NC_CACHE["nc"] = build_nc()
    return _NC_CACHE["nc"]


def make_in_maps(hidden_states, w1, w2, w3, langs):
    hs = np.asarray(hidden_states, np.float32)
    w1 = np.asarray(w1, np.float32)
    w2 = np.asarray(w2, np.float32)
    w3 = np.asarray(w3, np.float32)
    langs = np.asarray(langs)
    DT, FT = D // P, F // P
    in_maps = []
    for b in range(B):
        e = int(langs[b, 0] - LANG_BASE) % E
        xtb = np.ascontiguousarray(hs[b].T.astype(bf16)).reshape(DT, P, S)
        w1b = np.ascontiguousarray(
            w1[e].reshape(DT, P, FT, P).transpose(2, 1, 0, 3).astype(bf16))
        w3b = np.ascontiguousarray(
            w3[e].reshape(DT, P, FT, P).transpose(2, 1, 0, 3).astype(bf16))
        w2b = np.ascontiguousarray(
            w2[e].reshape(FT, P, DT, P).transpose(2, 1, 0, 3).astype(bf16))
        in_maps.append({"xt": xtb, "w1": w1b, "w3": w3b, "w2": w2b})
    return in_maps


def assemble_output(results):
    out = np.empty((B, S, D), np.float32)
    for b in range(B):
        out[b] = results[b]["ot"].reshape(D, S).T
    return out


def kernel(hidden_states, w1, w2, w3, langs, **kw):
    nc = _get_nc()
    in_maps = make_in_maps(hidden_states, w1, w2, w3, langs)
    res = run_bass_kernel_spmd(nc, in_maps, list(range(8)))
    return assemble_output(res.results)


if __name__ == "__main__":
    rng = np.random.default_rng(0)
    hs = rng.standard_normal((B, S, D)).astype(np.float32)
    w1_ = (rng.standard_normal((E, D, F)) / np.sqrt(D)).astype(np.float32)
    w3_ = (rng.standard_normal((E, D, F)) / np.sqrt(D)).astype(np.float32)
    w2_ = (rng.standard_normal((E, F, D)) / np.sqrt(F)).astype(np.float32)
    langs = rng.integers(4, 12, (B, 1)).astype(np.int64)
    out = kernel(hs, w1_, w2_, w3_, langs)
    print(out.shape, out.dtype)


# revision 19
# speedup vs baseline: 1.0116x; 1.0116x over previous
"""MBart expert-layer (MoE routing) kernel for 8 Trainium2 NeuronCores.

Strategy: data-parallel over batch. Each batch row routes to exactly one
expert (lang code), so the expert gather happens on host (langs are host
data).  Core b computes a dense SwiGLU MLP for row b:
    out = (gelu(x @ W1) * (x @ W3)) @ W2
All device work happens in transposed orientation (activations stored
[d_model, seq]) so both matmul stages use the natural [K, M] weight layouts
as the stationary operand and no on-device transposes are needed.
Matmul inputs are bf16 (fp32 accumulate in PSUM); gelu/mul in fp32.

The TPB ISA allows one sync wait per instruction and this walrus build
refuses multi-wait instructions, so the module is built as bacc.Bacc and
nc.compile() runs bacc's generate_event_semaphores pass, which splits
excess waits into event-semaphore chains.  build_nc() asserts the
resulting ≤1-wait invariant.
"""

import numpy as np
import ml_dtypes
from contextlib import ExitStack

import concourse.bass as bass
import concourse.bacc as bacc
import concourse.mybir as mybir
from concourse.tile import TileContext
from concourse.bass_utils import run_bass_kernel_spmd

E, B, S, D, F = 8, 8, 2048, 1024, 4096
LANG_BASE = 4
P = 128
MT = 512          # matmul moving free dim (seq chunk)
NG = 2            # seq super-chunks; weights streamed NG times
BF16 = mybir.dt.bfloat16
F32 = mybir.dt.float32
bf16 = ml_dtypes.bfloat16


def build_nc(S_=S, D_=D, F_=F, MT_=MT, NG_=NG,
             act=mybir.ActivationFunctionType.Gelu, check_waits=True):
    DT, FT = D_ // P, F_ // P
    sg = S_ // NG_
    nm = sg // MT_
    W2SUB = 4 if FT % 4 == 0 else 1   # w2 block split into sub-DMAs per d_i
    FS = FT // W2SUB                  # f-tiles per w2 sub-block
    nc = bacc.Bacc()
    xt = nc.declare_dram_parameter("xt", [DT, P, S_], BF16, isOutput=False)
    w1 = nc.declare_dram_parameter("w1", [FT, P, DT, P], BF16, isOutput=False)
    w3 = nc.declare_dram_parameter("w3", [FT, P, DT, P], BF16, isOutput=False)
    w2 = nc.declare_dram_parameter("w2", [DT, P, FT, P], BF16, isOutput=False)
    ot = nc.declare_dram_parameter("ot", [DT, P, S_], F32, isOutput=True)

    with TileContext(nc) as tc, ExitStack() as ctx:
        xpool = ctx.enter_context(tc.tile_pool(name="x", bufs=1))
        wpool = ctx.enter_context(tc.tile_pool(name="w", bufs=4))
        w2pool = ctx.enter_context(tc.tile_pool(name="w2", bufs=2))
        hpool = ctx.enter_context(tc.tile_pool(name="h", bufs=1))
        gpool = ctx.enter_context(tc.tile_pool(name="g", bufs=3))
        opool = ctx.enter_context(tc.tile_pool(name="o", bufs=3))
        ppool = ctx.enter_context(tc.tile_pool(name="ps", bufs=2, space="PSUM"))
        p2pool = ctx.enter_context(tc.tile_pool(name="ps2", bufs=2, space="PSUM"))

        x_sb = []
        for d_i in range(DT):
            t = xpool.tile([P, S_], BF16, name=f"x{d_i}", tag=f"x{d_i}")
            x_sb.append(t)
        # Load the first super-chunk's columns of every x tile before the
        # rest so the first matmuls stop waiting on the full 4MB transfer.
        for g in range(NG_):
            for d_i in range(DT):
                nc.sync.dma_start(
                    out=x_sb[d_i][:, g * sg:(g + 1) * sg],
                    in_=xt[d_i][:, g * sg:(g + 1) * sg])

        for g in range(NG_):
            s0 = g * sg
            # ---- phase A: hT[f, m] = gelu(W1.T x) * (W3.T x) ----
            h_tiles = []
            for f_i in range(FT):
                w1_t = wpool.tile([P, DT, P], BF16, name="w1t", tag="w1t")
                w3_t = wpool.tile([P, DT, P], BF16, name="w3t", tag="w3t")
                nc.gpsimd.dma_start(out=w1_t[:], in_=w1[f_i])
                nc.gpsimd.dma_start(out=w3_t[:], in_=w3[f_i])
                h_sb = hpool.tile([P, sg], BF16, name=f"h{f_i}", tag=f"h{f_i}")
                for m in range(nm):
                    ms = s0 + m * MT_
                    a_ps = ppool.tile([P, MT_], F32, name="a_ps", tag="a")
                    b_ps = ppool.tile([P, MT_], F32, name="b_ps", tag="b")
                    # b (w3 path) first so the gelu is the latest producer
                    # feeding the h-mul: the wait legalizer can then anchor
                    # the mul's PE wait on the gelu at zero cost.
                    for d_i in range(DT):
                        nc.tensor.matmul(
                            b_ps[:], w3_t[:, d_i, :], x_sb[d_i][:, ms:ms + MT_],
                            start=(d_i == 0), stop=(d_i == DT - 1))
                    for d_i in range(DT):
                        nc.tensor.matmul(
                            a_ps[:], w1_t[:, d_i, :], x_sb[d_i][:, ms:ms + MT_],
                            start=(d_i == 0), stop=(d_i == DT - 1))
                    g_sb = gpool.tile([P, MT_], F32, name="g_sb", tag="g")
                    nc.scalar.activation(g_sb[:], a_ps[:], act)
                    nc.vector.tensor_mul(
                        h_sb[:, m * MT_:(m + 1) * MT_], g_sb[:], b_ps[:])
                h_tiles.append(h_sb)
            # ---- phase B: outT[d, m] = W2.T hT ----
            for d_i in range(DT):
                w2_ts = []
                for k in range(W2SUB):
                    w2_t = w2pool.tile([P, FS, P], BF16, name=f"w2t{k}",
                                       tag=f"w2t{k}")
                    nc.gpsimd.dma_start(
                        out=w2_t[:], in_=w2[d_i][:, k * FS:(k + 1) * FS, :])
                    w2_ts.append(w2_t)
                for m in range(nm):
                    o_ps = p2pool.tile([P, MT_], F32, name="o_ps", tag="o")
                    for f_i in range(FT):
                        nc.tensor.matmul(
                            o_ps[:], w2_ts[f_i // FS][:, f_i % FS, :],
                            h_tiles[f_i][:, m * MT_:(m + 1) * MT_],
                            start=(f_i == 0), stop=(f_i == FT - 1))
                    o_sb = opool.tile([P, MT_], F32, name="o_sb", tag="osb")
                    nc.vector.tensor_copy(o_sb[:], o_ps[:])
                    nc.sync.dma_start(
                        out=ot[d_i][:, s0 + m * MT_:s0 + (m + 1) * MT_],
                        in_=o_sb[:])

    nc.compile()
    if check_waits:
        skip = ("InstDrain", "InstEventSemaphore")
        bad = []
        for f in nc.m.functions:
            for bb in f.blocks:
                for inst in bb.instructions:
                    if type(inst).__name__ in skip or inst.sync_info is None:
                        continue
                    nw = len(inst.sync_info.on_wait or [])
                    if nw > 1:
                        bad.append((inst.name, type(inst).__name__, nw))
        if bad:
            raise RuntimeError(f"insts with >1 wait: {bad[:8]}")
    return nc


_NC_CACHE = {}


def _get_nc():
    if "nc" not in _NC_CACHE:
        _NC_CACHE["nc"] = build_nc()
    return _NC_CACHE["nc"]


def make_in_maps(hidden_states, w1, w2, w3, langs):
    hs = np.asarray(hidden_states, np.float32)
    w1 = np.asarray(w1, np.float32)
    w2 = np.asarray(w2, np.float32)
    w3 = np.asarray(w3, np.float32)
    langs = np.asarray(langs)
    DT, FT = D // P, F // P
    in_maps = []
    for b in range(B):
        e = int(langs[b, 0] - LANG_BASE) % E
        xtb = np.ascontiguousarray(hs[b].T.astype(bf16)).reshape(DT, P, S)
        w1b = np.ascontiguousarray(
            w1[e].reshape(DT, P, FT, P).transpose(2, 1, 0, 3).astype(bf16))
        w3b = np.ascontiguousarray(
            w3[e].reshape(DT, P, FT, P).transpose(2, 1, 0, 3).astype(bf16))
        w2b = np.ascontiguousarray(
            w2[e].reshape(FT, P, DT, P).transpose(2, 1, 0, 3).astype(bf16))
        in_maps.append({"xt": xtb, "w1": w1b, "w3": w3b, "w2": w2b})
    return in_maps


def assemble_output(results):
    out = np.empty((B, S, D), np.float32)
    for b in range(B):
        out[b] = results[b]["ot"].reshape(D, S).T
    return out


def kernel(hidden_states, w1, w2, w3, langs, **kw):
    nc = _get_nc()
    in_maps = make_in_maps(hidden_states, w1, w2, w3, langs)
    res = run_bass_kernel_spmd(nc, in_maps, list(range(8)))
    return assemble_output(res.results)


if __name__ == "__main__":
    rng = np.random.default_rng(0)
    hs = rng.standard_normal((B, S, D)).astype(np.float32)
    w1_ = (rng.standard_normal((E, D, F)) / np.sqrt(D)).astype(np.float32)
    w3_ = (rng.standard_normal((E, D, F)) / np.sqrt(D)).astype(np.float32)
    w2_ = (rng.standard_normal((E, F, D)) / np.sqrt(F)).astype(np.float32)
    langs = rng.integers(4, 12, (B, 1)).astype(np.int64)
    out = kernel(hs, w1_, w2_, w3_, langs)
    print(out.shape, out.dtype)
